# revision 1
# baseline (speedup 1.0000x reference)
"""Trainium2 Bass kernel for nn_NERModel loss (CE + quadruplet + context MSE).

Strategy (8 NeuronCores, data-parallel over batch):
  - Each core processes 8 batches = 8192 tokens of embeddings [8192, 384] f32.
  - Tokens are tiled 128/tile at stride 127 (65 tiles) so every adjacent-token
    pair falls inside some tile; host-built per-(tile,slot) weights de-dup
    overlapping tokens/pairs exactly once.
  - CE: PE transposes emb chunks (PSUM), ScE/VE copy to SBUF, then
    logitsT[17,512] = W.T-chunk (lhsT) @ embT (rhs) accumulated over 3 K-chunks.
    exp on ScE with per-partition bias=b (free bias add), per-token sel via a
    fused tensor_tensor_reduce against a host-built ce_w-scaled one-hot, and
    sumexp column sums via an accumulating row-placement matmul into one
    persistent PSUM bank. One ln at the end.
  - CTX: one matmul per tile with constant (S - I) weights produces adjacent
    diffs straight into PSUM; fused square+pair-weight+row-reduce split across
    ScE (activation Square, scale=w, accum_out) and VE (tensor_tensor_reduce).
  - Device returns two partial sums per core; host does the tiny quadruplet
    term (index scans over labels + 49 gathered rows) and final combination.
"""

import sys

for _p in ("/opt/trn_rl_repo", "/root/.axon_site/_ro/trn_rl_repo"):
    if _p not in sys.path:
        sys.path.append(_p)

import numpy as np
from contextlib import ExitStack

import concourse.bass as bass
import concourse.bacc as bacc
import concourse.mybir as mybir
from concourse import tile
from concourse.ap import AP

NUM_LABELS = 17
MARGIN = 1.0
IGNORE = -100

B, S, H, L = 64, 1024, 384, NUM_LABELS
NCORES = 8
BP = B // NCORES            # batches per core
NTOK = BP * S               # tokens per core (8192)
STRIDE = 127                # token stride between tiles (1-token overlap)
NT = 65                     # tiles per core
NG = (NT + 3) // 4          # compute groups of 4 tiles -> 17
GDMA = 8                    # tiles per DMA transfer
NDMA = (NT + GDMA - 1) // GDMA  # 9
F32 = mybir.dt.float32


def _tile_start(t: int) -> int:
    # last tile is clamped so it stays in-bounds; duplicated tokens/pairs are
    # zero-weighted on the host side
    return NTOK - 128 if t == NT - 1 else STRIDE * t


def _build_nc() -> bass.Bass:
    import os

    skip_ctx = bool(os.environ.get("NER_SKIP_CTX"))
    skip_ce = bool(os.environ.get("NER_SKIP_CE"))
    no_gpsimd = bool(os.environ.get("NER_NO_GPSIMD_MEMSET"))
    # Bacc (not plain Bass): its compile() legalizes sync waits (>=2 waits per
    # instruction are split / moved to LDWEIGHTS), which walrus requires.
    nc = bacc.Bacc("TRN2", debug=False)

    emb = nc.declare_dram_parameter("emb", [NTOK, H], F32, isOutput=False)
    woh = nc.declare_dram_parameter("woh", [L, NG * 512], F32, isOutput=False)
    cewg = nc.declare_dram_parameter("cewg", [NG, 512], F32, isOutput=False)
    pairw = nc.declare_dram_parameter("pairw", [128, NT], F32, isOutput=False)
    wt = nc.declare_dram_parameter("wt", [128, 3 * L], F32, isOutput=False)
    bcol = nc.declare_dram_parameter("bcol", [L, 1], F32, isOutput=False)
    selg = nc.declare_dram_parameter("selg", [L, NG * L], F32, isOutput=False)
    dfw = nc.declare_dram_parameter("dfw", [128, 128], F32, isOutput=False)
    idn = nc.declare_dram_parameter("idn", [128, 128], F32, isOutput=False)
    ones = nc.declare_dram_parameter("ones", [128, 1], F32, isOutput=False)
    outv = nc.declare_dram_parameter("outv", [1, 8], F32, isOutput=True)

    AF = mybir.ActivationFunctionType
    AX = mybir.AxisListType
    OP = mybir.AluOpType

    with tile.TileContext(nc) as tc, ExitStack() as ctx:
        consts = ctx.enter_context(tc.tile_pool(name="consts", bufs=1))
        nat_pool = ctx.enter_context(tc.tile_pool(name="nat", bufs=3))
        embt_pool = ctx.enter_context(tc.tile_pool(name="embt", bufs=2))
        expt_pool = ctx.enter_context(tc.tile_pool(name="expt", bufs=2))
        junk_pool = ctx.enter_context(tc.tile_pool(name="junk", bufs=2))
        acc_pool = ctx.enter_context(tc.tile_pool(name="acc", bufs=1))
        ps_t = ctx.enter_context(tc.tile_pool(name="ps_t", bufs=1, space="PSUM"))
        ps_l = ctx.enter_context(tc.tile_pool(name="ps_l", bufs=1, space="PSUM"))
        ps_d = ctx.enter_context(tc.tile_pool(name="ps_d", bufs=1, space="PSUM"))
        ps_s = ctx.enter_context(tc.tile_pool(name="ps_s", bufs=1, space="PSUM"))

        def cload(handle, shape):
            t = consts.tile(list(shape), F32, tag=handle.name + "_c")
            nc.sync.dma_start(out=t[:], in_=handle.ap())
            return t

        woh_t = cload(woh, (L, NG * 512))
        cewg_t = cload(cewg, (NG, 512))
        pairw_t = cload(pairw, (128, NT))
        wt_t = cload(wt, (128, 3 * L))
        bcol_t = cload(bcol, (L, 1))
        selg_t = cload(selg, (L, NG * L))
        dfw_t = cload(dfw, (128, 128))
        idn_t = cload(idn, (128, 128))
        ones_t = cload(ones, (128, 1))

        # persistent accumulators
        sumexp_ps = ps_s.tile([L, 512], F32)          # [group, group-token]
        ctxbuf = acc_pool.tile([128, NT], F32)        # per-tile weighted ||diff||^2
        selbuf = acc_pool.tile([L, NG], F32)          # per-group sum cew*logit
        nc.vector.memset(selbuf[:], 0.0)

        nat_tiles = {}

        simple_dma = bool(os.environ.get("NER_SIMPLE_DMA"))
        skip_emb_dma = bool(os.environ.get("NER_SKIP_EMB_DMA"))

        def do_dma(d: int):
            ntl = min(GDMA, NT - d * GDMA)
            nat = nat_pool.tile([128, GDMA * H], F32, tag="natbuf")
            if skip_emb_dma:
                nat_tiles[d] = nat
                return
            if simple_dma:
                for j in range(ntl):
                    src = AP(
                        tensor=emb,
                        offset=_tile_start(d * GDMA + j) * H,
                        ap=[[H, 128], [1, H]],
                    )
                    nc.sync.dma_start(out=nat[:, j * H : (j + 1) * H], in_=src)
            elif ntl == GDMA:
                src = AP(
                    tensor=emb,
                    offset=_tile_start(d * GDMA) * H,
                    ap=[[H, 128], [STRIDE * H, GDMA], [1, H]],
                )
                nc.sync.dma_start(out=nat[:, :].rearrange("p (g h) -> p g h", h=H), in_=src)
            else:
                src = AP(
                    tensor=emb,
                    offset=_tile_start(d * GDMA) * H,
                    ap=[[H, 128], [1, H]],
                )
                nc.sync.dma_start(out=nat[:, 0:H], in_=src)
            nat_tiles[d] = nat

        def nat_slice(t: int, c0: int, c1: int):
            nat = nat_tiles[t // GDMA]
            base = (t % GDMA) * H
            return nat[:, base + c0 : base + c1]

        def _ctx_only(tiles):
            for half in range(2):
                tiles_h = tiles[2 * half : 2 * half + 2]
                if not tiles_h:
                    break
                df_ps = ps_d.tile([128, 2, 512], F32, tag="df_ps")
                for jj, t in enumerate(tiles_h):
                    nc.tensor.matmul(
                        df_ps[:, jj, 0:H], dfw_t[:], nat_slice(t, 0, H),
                        start=True, stop=True,
                    )
                _sqw(tiles_h, df_ps)

        def do_group(g: int):
            tiles = list(range(4 * g, min(4 * g + 4, NT)))
            last = len(tiles) < 4

            # ---- transposes: embT[h, tok] chunks ----
            if skip_ce:
                _ctx_only(tiles)
                return
            embT_ps = ps_t.tile([128, 3 * 512], F32, tag="embT_ps")
            for j, t in enumerate(tiles):
                for c in range(3):
                    # out = nat_chunk.T via a normal matmul against identity
                    # (transpose-mode LW has too few sync-wait slots in codegen)
                    nc.tensor.matmul(
                        embT_ps[:, c * 512 + j * 128 : c * 512 + (j + 1) * 128],
                        nat_slice(t, c * 128, (c + 1) * 128),
                        idn_t[:],
                        start=True,
                        stop=True,
                    )
            embT = embt_pool.tile([128, 3 * 512], F32, tag="embT")
            if last:
                # only j=0 columns are real; zero the rest so downstream
                # full-width ops read finite garbage
                (nc.vector if no_gpsimd else nc.gpsimd).memset(embT[:], 0.0)
                ev = embT[:, :].rearrange("p (c k) -> p c k", k=512)
                pv = embT_ps[:, :].rearrange("p (c k) -> p c k", k=512)
                nc.vector.tensor_copy(ev[:, :, 0:128], pv[:, :, 0:128])
            else:
                nc.vector.tensor_copy(embT[:], embT_ps[:])

            # ---- logitsT [17, 512] ----
            lg_ps = ps_l.tile([L, 512], F32, tag="lg_ps")
            for c in range(3):
                nc.tensor.matmul(
                    lg_ps[:],
                    wt_t[:, c * L : (c + 1) * L],
                    embT[:, c * 512 : (c + 1) * 512],
                    start=(c == 0),
                    stop=(c == 2),
                )

            # ---- exp(logit + b) ----
            expT = expt_pool.tile([L, 512], F32, tag="expT")
            nc.scalar.activation(expT[:], lg_ps[:], AF.Exp, bias=bcol_t[:, 0:1], scale=1.0)

            # ---- sel accumulation: selacc += sum(logit * woh) ----
            junk17 = junk_pool.tile([L, 512], F32, tag="junk17")
            nc.vector.tensor_mul(junk17[:], lg_ps[:], woh_t[:, g * 512 : (g + 1) * 512])
            junk17c = junk_pool.tile([L, 512], F32, tag="junk17b")
            nc.vector.tensor_scalar(
                out=junk17c[:], in0=junk17[:], scalar1=1.0, scalar2=None,
                op0=OP.mult, op1=OP.add, accum_out=selbuf[:, g : g + 1],
            )

            # ---- sumexp row-placement matmul ----
            nc.tensor.matmul(
                sumexp_ps[:],
                selg_t[:, g * L : (g + 1) * L],
                expT[:],
                start=(g == 0),
                stop=(g == NG - 1),
            )

            # ---- ctx: diff = emb[t+1]-emb[t] via (S-I) matmul, then w*||diff||^2 ----
            # each matmul output must live inside one 512-col PSUM bank, so
            # pad each tile's diff region to 512 and process 2 tiles per alloc
            if skip_ctx:
                return
            for half in range(2):
                tiles_h = tiles[2 * half : 2 * half + 2]
                if not tiles_h:
                    break
                df_ps = ps_d.tile([128, 2, 512], F32, tag="df_ps")
                for jj, t in enumerate(tiles_h):
                    nc.tensor.matmul(
                        df_ps[:, jj, 0:H],
                        dfw_t[:],
                        nat_slice(t, 0, H),
                        start=True,
                        stop=True,
                    )
                _sqw(tiles_h, df_ps)

        def _sqw(tiles_h, df_ps):
            for jj, t in enumerate(tiles_h):
                dsl = df_ps[:, jj, 0:H]
                if False:
                    pass
                else:
                    jk = junk_pool.tile([128, H], F32, tag="junkS")
                    nc.scalar.activation(
                        jk[:],
                        dsl,
                        AF.Square,
                        bias=0.0,
                        scale=pairw_t[:, t : t + 1],
                        accum_out=ctxbuf[:, t : t + 1],
                    )

        g_done = 0
        for d in range(NDMA):
            do_dma(d)
            # run all compute groups fully covered by the DMAs issued so far
            tiles_ready = min((d + 1) * GDMA, NT)
            while g_done < NG and min(4 * g_done + 4, NT) <= tiles_ready:
                do_group(g_done)
                g_done += 1
        assert g_done == NG

        # ---- final reduction ----
        skip_final = bool(os.environ.get("NER_SKIP_FINAL"))
        if skip_final:
            outs0 = acc_pool.tile([1, 8], F32)
            nc.vector.memset(outs0[:], 0.0)
            nc.sync.dma_start(out=outv.ap(), in_=outs0[:])
        if skip_ce:
            nc.vector.memset(sumexp_ps[:], 1.0)
        if skip_ctx:
            nc.vector.memset(ctxbuf[:], 0.0)
        if not skip_final:
            lnsum = expt_pool.tile([L, 512], F32, tag="lnsum")
            nc.scalar.activation(lnsum[:], sumexp_ps[:], AF.Ln)
            accA = acc_pool.tile([L, 1], F32)
            junk17b = junk_pool.tile([L, 512], F32, tag="junk17")
            nc.vector.tensor_mul(junk17b[:], lnsum[:], cewg_t[:])
            junk17d = junk_pool.tile([L, 512], F32, tag="junk17b")
            nc.vector.tensor_scalar(
                out=junk17d[:], in0=junk17b[:], scalar1=1.0, scalar2=None,
                op0=OP.mult, op1=OP.add, accum_out=accA[:, 0:1],
            )
            selacc = acc_pool.tile([L, 1], F32)
            junkS = junk_pool.tile([L, NG], F32, tag="junkS17")
            nc.vector.tensor_scalar(
                out=junkS[:], in0=selbuf[:], scalar1=1.0, scalar2=None,
                op0=OP.mult, op1=OP.add, accum_out=selacc[:, 0:1],
            )
            cev = acc_pool.tile([L, 1], F32)
            nc.vector.tensor_sub(cev[:], accA[:], selacc[:])
            fin1 = ps_l.tile([1, 1], F32, tag="lg_ps")
            nc.tensor.matmul(fin1[:], cev[:], ones_t[0:L, :], start=True, stop=True)

            ctxsum = acc_pool.tile([128, 1], F32)
            nc.vector.tensor_reduce(ctxsum[:], ctxbuf[:], axis=AX.X, op=OP.add)
            fin2 = ps_l.tile([1, 1], F32, tag="lg_ps")
            nc.tensor.matmul(fin2[:], ctxsum[:], ones_t[:], start=True, stop=True)

            outs = acc_pool.tile([1, 8], F32)
            nc.vector.memset(outs[:], 0.0)
            nc.scalar.copy(outs[0:1, 0:1], fin1[:])
            nc.scalar.copy(outs[0:1, 1:2], fin2[:])
            nc.sync.dma_start(out=outv.ap(), in_=outs[:])

    nc.compile()
    return nc


# ---------------------------------------------------------------------------
# host-side preparation


def _host_grids(labf: np.ndarray, mskf: np.ndarray):
    """Per-core grids. labf/mskf: [NTOK] int64/int32.

    Returns (cew_grid [NT,128], pairw_grid [NT,128], woh [L, NG*512],
             cewg [NG, 512])."""
    valid = labf != IGNORE
    pair_ok = np.zeros(NTOK, dtype=bool)
    lf = labf.astype(np.int64)
    # pair (k, k+1) within a batch row of length S
    k = np.arange(NTOK - 1)
    in_batch = (k % S) != (S - 1)
    pair_ok[:-1] = in_batch & (lf[:-1] != IGNORE) & (lf[:-1] == lf[1:]) & (lf[:-1] > 0)

    cew_grid = np.zeros((NT, 128), np.float32)
    pairw_grid = np.zeros((NT, 128), np.float32)
    seen_tok = np.zeros(NTOK, dtype=bool)
    seen_pair = np.zeros(NTOK, dtype=bool)
    tokmap = np.zeros((NT, 128), np.int64)
    for t in range(NT):
        s0 = _tile_start(t)
        toks = np.arange(s0, s0 + 128)
        tokmap[t] = toks
        fresh = ~seen_tok[toks]
        cew_grid[t] = (valid[toks] & fresh).astype(np.float32)
        seen_tok[toks] = True
        pfresh = ~seen_pair[toks]
        pw = pair_ok[toks] & pfresh
        pw[127] = False  # col 127 diff is garbage by construction
        pairw_grid[t] = pw.astype(np.float32)
        seen_pair[toks[:127]] = True

    woh = np.zeros((L, NG * 512), np.float32)
    cewg = np.zeros((NG, 512), np.float32)
    for g in range(NG):
        for j in range(min(4, NT - 4 * g)):
            t = 4 * g + j
            toks = tokmap[t]
            cols = g * 512 + j * 128 + np.arange(128)
            cewg[g, j * 128 : (j + 1) * 128] = cew_grid[t]
            lab_c = np.where(valid[toks], lf[toks], 0)
            woh[lab_c, cols] = cew_grid[t]
    return cew_grid, pairw_grid, woh, cewg


def _quad_host(fe: np.ndarray, fl: np.ndarray, fm: np.ndarray) -> np.float32:
    """Mirror of the reference quadruplet loss in numpy float32."""
    N = fe.shape[0]
    idx = np.arange(N, dtype=np.int64)
    BIG = N
    fm_b = fm > 0
    is_ent = fm_b & (fl > 0)
    non_ent = fm_b & (fl == 0)
    d_i = np.min(np.where(non_ent, idx, BIG))
    has_non = bool(non_ent.any())

    a_i = np.zeros(L - 1, np.int64)
    p_i = np.zeros(L - 1, np.int64)
    n_i = np.zeros(L - 1, np.int64)
    ok = np.zeros(L - 1, bool)
    for i, t in enumerate(range(1, L)):
        m = is_ent & (fl == t)
        order = np.sort(np.where(m, idx, BIG))
        a_i[i], p_i[i] = order[0], order[1]
        cnt = int(m.sum())
        other = is_ent & (fl != t)
        n_i[i] = np.min(np.where(other, idx, BIG))
        ok[i] = (cnt >= 2) and bool(other.any()) and has_non

    clip = lambda v: np.clip(v, 0, N - 1)
    A = fe[clip(a_i)]
    P = fe[clip(p_i)]
    Ng = fe[clip(n_i)]
    D = fe[clip(np.array([d_i]))]
    eps = np.float32(1e-6)

    def dist(x, y):
        d = (x - y + eps).astype(np.float32)
        return np.sqrt(np.sum(d * d, axis=-1, dtype=np.float32)).astype(np.float32)

    pd, nd, dd = dist(A, P), dist(A, Ng), dist(A, D)
    ql = np.maximum(pd - nd + np.float32(MARGIN), 0) + np.maximum(
        pd - dd + np.float32(2.0 * MARGIN), 0
    )
    qcnt = int(ok.sum())
    quad = float(np.sum(np.where(ok, ql, 0.0), dtype=np.float64)) / max(qcnt, 1)
    return np.float32(quad if qcnt > 0 else 0.0)


_NC_CACHE = {}


def _get_nc():
    if "nc" not in _NC_CACHE:
        _NC_CACHE["nc"] = _build_nc()
    return _NC_CACHE["nc"]


def _device_consts():
    if "consts" in _NC_CACHE:
        return _NC_CACHE["consts"]
    dfw = np.zeros((128, 128), np.float32)
    for t in range(127):
        dfw[t + 1, t] = 1.0
    dfw[np.arange(128), np.arange(128)] -= 1.0
    idn = np.eye(128, dtype=np.float32)
    ones = np.ones((128, 1), np.float32)
    selg = np.zeros((L, NG * L), np.float32)
    for g in range(NG):
        selg[:, g * L + g] = 1.0
    _NC_CACHE["consts"] = (dfw, idn, ones, selg)
    return _NC_CACHE["consts"]


def kernel(embeddings, classifier_w, classifier_b, labels, attention_mask):
    from concourse.bass_utils import run_bass_kernel_spmd

    emb = np.ascontiguousarray(np.asarray(embeddings, dtype=np.float32))
    W = np.asarray(classifier_w, dtype=np.float32)
    b = np.asarray(classifier_b, dtype=np.float32)
    lab = np.asarray(labels)
    msk = np.asarray(attention_mask)

    lab_f = lab.reshape(-1).astype(np.int64)
    msk_f = msk.reshape(-1).astype(np.int64)
    N = B * S

    wt = np.zeros((128, 3 * L), np.float32)
    for c in range(3):
        wt[:, c * L : (c + 1) * L] = W[:, c * 128 : (c + 1) * 128].T
    bcol = b.reshape(L, 1).astype(np.float32)
    dfw, idn, ones, selg = _device_consts()

    in_maps = []
    cew_grids = []
    for cidx in range(NCORES):
        sl = slice(cidx * NTOK, (cidx + 1) * NTOK)
        labc = lab_f[sl]
        cewg_grid, pairw_grid, woh, cewg = _host_grids(labc, msk_f[sl])
        cew_grids.append(cewg_grid)
        in_maps.append(
            {
                "emb": emb.reshape(N, H)[sl],
                "woh": woh,
                "cewg": cewg,
                "pairw": np.ascontiguousarray(pairw_grid.T),
                "wt": wt,
                "bcol": bcol,
                "selg": selg,
                "dfw": dfw,
                "idn": idn,
                "ones": ones,
            }
        )

    nc = _get_nc()
    res = run_bass_kernel_spmd(nc, in_maps, list(range(NCORES)))

    ce_sum = 0.0
    ctx_sum = 0.0
    for cidx in range(NCORES):
        out = res.results[cidx]["outv"]
        ce_sum += float(out[0, 0])
        ctx_sum += float(out[0, 1])

    valid = lab_f != IGNORE
    ce_cnt = int(valid.sum())
    # device sel used logits without bias; correct with sum(cew * b[label])
    lab_safe = np.where(valid, lab_f, 0)
    ce_sum -= float(np.sum(np.where(valid, b[lab_safe], 0.0), dtype=np.float64))
    ce = ce_sum / max(ce_cnt, 1)

    pair_ok = np.zeros(N, dtype=bool)
    k = np.arange(N - 1)
    in_batch = (k % S) != (S - 1)
    pair_ok[:-1] = (
        in_batch & (lab_f[:-1] != IGNORE) & (lab_f[:-1] == lab_f[1:]) & (lab_f[:-1] > 0)
    )
    pc = int(pair_ok.sum())
    ctx = (ctx_sum / H) / max(pc, 1) if pc > 0 else 0.0

    quad = _quad_host(emb.reshape(N, H), lab_f, msk_f)

    loss = ce + 0.5 * float(quad) + 0.1 * ctx
    return np.float32(loss)



# revision 12
# speedup vs baseline: 3.1044x; 3.1044x over previous
"""Trainium2 Bass kernel for nn_NERModel loss (CE + quadruplet + context MSE).

v3 strategy (8 NeuronCores, data-parallel over batch):
  - Host converts embeddings to bf16 -> DMA volume halves (6.3 MB/core).
  - embT (h on partitions, tokens on the free axis) is produced directly by
    the DMA crossbar transpose (dma_start_transpose) while loading from
    DRAM: 4 blocks x 3 h-chunks of [2048 tok, 128 h] -> [128, 2048].
    No PE transposes, no PSUM staging, no SBUF copies.
  - CE: logitsT[17,512] per group of 512 tokens, 4 groups stacked per PSUM
    bank at partition 32*j; exp on ScE per block; per-(group,token) sumexp
    via one row-placement matmul per block into a persistent bank;
    selected-logit sum via mul + tensor_scalar-accumulate against a
    host-built one-hot (tensor_tensor_reduce crashes TRN2 - do not use);
    single Ln + weighted reduces at the end.
  - CTX: adjacent-token diffs are adjacent columns of embT -> one strided
    DVE subtract + one square per group (bf16), then a row-placement
    matmul accumulates per-(group,pair) ||diff||^2 into a persistent
    [16,512] PSUM bank; host-built 0/1 pair weights applied at the end.
    Block-boundary pairs (3/core) are added on host.
  - Device returns two partial sums per core; host does the tiny
    quadruplet term and final combination.
"""

import os
import sys

for _p in ("/opt/trn_rl_repo", "/root/.axon_site/_ro/trn_rl_repo"):
    if _p not in sys.path:
        sys.path.append(_p)

import numpy as np
import ml_dtypes
from contextlib import ExitStack

import concourse.bass as bass
import concourse.bacc as bacc
import concourse.mybir as mybir
from concourse import tile
from concourse.ap import AP

NUM_LABELS = 17
MARGIN = 1.0
IGNORE = -100

B, S, H, L = 64, 1024, 384, NUM_LABELS
NCORES = 8
BP = B // NCORES            # batches per core
NTOK = BP * S               # tokens per core (8192)
NG = 16                     # groups of 512 tokens
NB = 4                      # blocks of 4 groups (2048 tokens)
F32 = mybir.dt.float32
BF16 = mybir.dt.bfloat16
BF = ml_dtypes.bfloat16


def _build_nc() -> bass.Bass:
    # how many groups' squares run on ScE (rest on VE) - rebalance knob
    sq_act = int(os.environ.get("NER_SQ_ACT", "12"))

    nc = bacc.Bacc("TRN2", debug=False)

    emb = nc.declare_dram_parameter("emb", [NTOK, H], BF16, isOutput=False)
    wt = nc.declare_dram_parameter("wt", [128, 3 * L], BF16, isOutput=False)
    selg4 = nc.declare_dram_parameter("selg4", [128, 4], BF16, isOutput=False)
    egall = nc.declare_dram_parameter("egall", [128, NG * NG], BF16, isOutput=False)
    bcol = nc.declare_dram_parameter("bcol", [128, 1], F32, isOutput=False)
    woh = nc.declare_dram_parameter("woh", [128, NB * 512], BF16, isOutput=False)
    cews = nc.declare_dram_parameter("cews", [128, 512], F32, isOutput=False)
    pairw = nc.declare_dram_parameter("pairw", [NG, 512], F32, isOutput=False)
    ones = nc.declare_dram_parameter("ones", [128, 1], F32, isOutput=False)
    outv = nc.declare_dram_parameter("outv", [1, 8], F32, isOutput=True)

    AF = mybir.ActivationFunctionType
    AX = mybir.AxisListType
    OP = mybir.AluOpType

    with tile.TileContext(nc) as tc, ExitStack() as ctx:
        consts = ctx.enter_context(tc.tile_pool(name="consts", bufs=1))
        embt_pool = ctx.enter_context(tc.tile_pool(name="embt", bufs=4))
        d_pool = ctx.enter_context(tc.tile_pool(name="dbuf", bufs=2))
        sq_pool = ctx.enter_context(tc.tile_pool(name="sqbuf", bufs=2))
        expt_pool = ctx.enter_context(tc.tile_pool(name="expt", bufs=2))
        junk_pool = ctx.enter_context(tc.tile_pool(name="junk", bufs=2))
        acc_pool = ctx.enter_context(tc.tile_pool(name="acc", bufs=1))
        ps_l = ctx.enter_context(tc.tile_pool(name="ps_l", bufs=2, space="PSUM"))
        ps_a = ctx.enter_context(tc.tile_pool(name="ps_a", bufs=1, space="PSUM"))
        ps_c = ctx.enter_context(tc.tile_pool(name="ps_c", bufs=1, space="PSUM"))

        def cload(handle, shape, dtype):
            t = consts.tile(list(shape), dtype, tag=handle.name + "_c")
            nc.sync.dma_start(out=t[:], in_=handle.ap())
            return t

        wt_t = cload(wt, (128, 3 * L), BF16)
        selg4_t = cload(selg4, (128, 4), BF16)
        egall_t = cload(egall, (128, NG * NG), BF16)
        bcol_t = cload(bcol, (128, 1), F32)
        woh_t = cload(woh, (128, NB * 512), BF16)
        cews_t = cload(cews, (128, 512), F32)
        pairw_t = cload(pairw, (NG, 512), F32)
        ones_t = cload(ones, (128, 1), F32)

        # persistent accumulators
        bankA = ps_a.tile([128, 512], F32)      # per-(group,token) sumexp
        nc.vector.memset(bankA[:], 1.0)         # ln(1)=0 on unused rows
        ctxps = ps_c.tile([NG, 512], F32)       # per-(group,pair) ||diff||^2
        lg = [
            ps_l.tile([128, 512], F32, tag="lg", name=f"lgbank{i}") for i in range(2)
        ]
        nc.vector.memset(lg[0][:], 0.0)         # exp(0)=1 on unused rows,
        nc.vector.memset(lg[1][:], 0.0)         # zeroed by selg4/woh
        selbuf = acc_pool.tile([128, NB], F32)

        embt_blks = {}

        def do_dma(blk: int):
            embT = embt_pool.tile([128, 3, 2048], BF16, tag="embTblk")
            for c in range(3):
                src = AP(
                    tensor=emb,
                    offset=(blk * 2048) * H + c * 128,
                    ap=[[H, 2048], [1, 128]],
                )
                nc.sync.dma_start_transpose(embT[:, c, :], src)
            embt_blks[blk] = embT

        def do_group(g: int):
            b, j = g // 4, g % 4
            embT = embt_blks[b]
            koff = 512 * j
            w = 512 if j < 3 else 511   # last in-block pair is block-boundary

            # ---- logits into lg[b%2] rows 32j..32j+16 ----
            lgb = lg[b % 2]
            for c in range(3):
                nc.tensor.matmul(
                    lgb[32 * j : 32 * j + L, :],
                    wt_t[:, c * L : (c + 1) * L],
                    embT[:, c, koff : koff + 512],
                    start=(c == 0), stop=(c == 2),
                    tile_position=(0, 32 * j),
                )

            # ---- ctx: d = embT[:, :, k+1] - embT[:, :, k]; sq = d*d ----
            dt = d_pool.tile([128, 3, 512], BF16, tag="dt")
            nc.vector.tensor_sub(
                dt[:, :, :w], embT[:, :, koff + 1 : koff + 1 + w],
                embT[:, :, koff : koff + w],
            )
            sq = sq_pool.tile([128, 3, 512], BF16, tag="sq")
            if g % NG < sq_act:
                nc.scalar.activation(sq[:, :, :w], dt[:, :, :w], AF.Square)
            else:
                nc.vector.tensor_mul(sq[:, :, :w], dt[:, :, :w], dt[:, :, :w])
            for c in range(3):
                nc.tensor.matmul(
                    ctxps[:, :w],
                    egall_t[:, g * NG : (g + 1) * NG],
                    sq[:, c, :w],
                    start=(g == 0 and c == 0), stop=(g == NG - 1 and c == 2),
                )

            # ---- block postprocess after last group of block ----
            if j == 3:
                ex = expt_pool.tile([128, 512], BF16, tag="ex")
                nc.scalar.activation(
                    ex[:], lgb[:], AF.Exp, bias=bcol_t[:, 0:1], scale=1.0
                )
                nc.tensor.matmul(
                    bankA[32 * b : 32 * b + 4, :], selg4_t[:], ex[:],
                    start=True, stop=True, tile_position=(0, 32 * b),
                )
                jt = junk_pool.tile([128, 512], F32, tag="jt")
                nc.vector.tensor_mul(
                    jt[:], lgb[:], woh_t[:, b * 512 : (b + 1) * 512]
                )
                jt2 = junk_pool.tile([128, 512], F32, tag="jt")
                nc.vector.tensor_scalar(
                    out=jt2[:], in0=jt[:], scalar1=1.0, scalar2=None,
                    op0=OP.mult, op1=OP.add,
                    accum_out=selbuf[:, b : b + 1],
                )

        for blk in range(NB):
            do_dma(blk)
            for j in range(4):
                do_group(4 * blk + j)

        # ---- final reduction ----
        lnsum = acc_pool.tile([128, 512], F32)
        nc.scalar.activation(lnsum[:], bankA[:], AF.Ln)
        acc1 = acc_pool.tile([128, 1], F32)
        jf1 = junk_pool.tile([128, 512], F32, tag="jt")
        nc.vector.tensor_mul(jf1[:], lnsum[:], cews_t[:])
        jf2 = junk_pool.tile([128, 512], F32, tag="jt")
        nc.vector.tensor_scalar(
            out=jf2[:], in0=jf1[:], scalar1=1.0, scalar2=None,
            op0=OP.mult, op1=OP.add, accum_out=acc1[:, 0:1],
        )
        selsum = acc_pool.tile([128, 1], F32)
        nc.vector.tensor_reduce(selsum[:], selbuf[:], axis=AX.X, op=OP.add)
        cev = acc_pool.tile([128, 1], F32)
        nc.vector.tensor_sub(cev[:], acc1[:], selsum[:])
        fin1 = ps_l.tile([1, 1], F32, tag="lg", name="fin1")
        nc.tensor.matmul(fin1[:], cev[:], ones_t[:], start=True, stop=True)

        junkC = acc_pool.tile([NG, 512], F32)
        acc3 = acc_pool.tile([NG, 1], F32)
        nc.vector.tensor_mul(junkC[:], ctxps[:], pairw_t[:])
        junkD = acc_pool.tile([NG, 512], F32)
        nc.vector.tensor_scalar(
            out=junkD[:], in0=junkC[:], scalar1=1.0, scalar2=None,
            op0=OP.mult, op1=OP.add, accum_out=acc3[:, 0:1],
        )
        fin2 = ps_l.tile([1, 1], F32, tag="lg", name="fin2")
        nc.tensor.matmul(fin2[:], acc3[:], ones_t[0:NG, :], start=True, stop=True)

        outs = acc_pool.tile([1, 8], F32)
        nc.vector.memset(outs[:], 0.0)
        nc.scalar.copy(outs[0:1, 0:1], fin1[:])
        nc.scalar.copy(outs[0:1, 1:2], fin2[:])
        nc.sync.dma_start(out=outv.ap(), in_=outs[:])

    nc.compile()
    return nc


# ---------------------------------------------------------------------------
# host-side preparation


def _host_tables(labf: np.ndarray):
    """Per-core CE/ctx weight tables. labf: [NTOK] int64.

    Row layouts match the device PSUM stacking:
      lg rows 32*j + l  (j = group-within-block, l = label)
      bankA rows 32*b + j  (b = block, j = group-within-block)
      ctxps rows g (group), cols k: pair (512g+k, 512g+k+1)
    """
    valid = labf != IGNORE
    lf = labf.astype(np.int64)
    t = np.arange(NTOK)
    g = t // 512
    k = t % 512
    b_blk = g // 4
    j_grp = g % 4

    woh = np.zeros((128, NB * 512), np.float32)
    lab_c = np.where(valid, lf, 0)
    rows = 32 * j_grp + lab_c
    cols = b_blk * 512 + k
    woh[rows[valid], cols[valid]] = 1.0

    cews = np.zeros((128, 512), np.float32)
    cews[32 * b_blk[valid] + j_grp[valid], k[valid]] = 1.0

    pair_ok = np.zeros(NTOK, dtype=bool)
    kk = np.arange(NTOK - 1)
    in_batch = (kk % S) != (S - 1)
    pair_ok[:-1] = in_batch & (lf[:-1] != IGNORE) & (lf[:-1] == lf[1:]) & (lf[:-1] > 0)
    pairw = np.zeros((NG, 512), np.float32)
    m = np.ones(NTOK, dtype=bool)
    m[-1] = False                       # no pair after last token
    m &= (t % 2048) != 2047             # block-boundary pairs done on host
    pairw[g[m], k[m]] = pair_ok[m].astype(np.float32)

    return woh.astype(BF), cews, pairw


def _quad_host(fe: np.ndarray, fl: np.ndarray, fm: np.ndarray) -> np.float32:
    """Mirror of the reference quadruplet loss in numpy float32."""
    N = fe.shape[0]
    idx = np.arange(N, dtype=np.int64)
    BIG = N
    fm_b = fm > 0
    is_ent = fm_b & (fl > 0)
    non_ent = fm_b & (fl == 0)
    d_i = np.min(np.where(non_ent, idx, BIG))
    has_non = bool(non_ent.any())

    a_i = np.zeros(L - 1, np.int64)
    p_i = np.zeros(L - 1, np.int64)
    n_i = np.zeros(L - 1, np.int64)
    ok = np.zeros(L - 1, bool)
    for i, ty in enumerate(range(1, L)):
        m = is_ent & (fl == ty)
        order = np.sort(np.where(m, idx, BIG))
        a_i[i], p_i[i] = order[0], order[1]
        cnt = int(m.sum())
        other = is_ent & (fl != ty)
        n_i[i] = np.min(np.where(other, idx, BIG))
        ok[i] = (cnt >= 2) and bool(other.any()) and has_non

    clip = lambda v: np.clip(v, 0, N - 1)
    A = fe[clip(a_i)]
    P = fe[clip(p_i)]
    Ng = fe[clip(n_i)]
    D = fe[clip(np.array([d_i]))]
    eps = np.float32(1e-6)

    def dist(x, y):
        d = (x - y + eps).astype(np.float32)
        return np.sqrt(np.sum(d * d, axis=-1, dtype=np.float32)).astype(np.float32)

    pd, nd, dd = dist(A, P), dist(A, Ng), dist(A, D)
    ql = np.maximum(pd - nd + np.float32(MARGIN), 0) + np.maximum(
        pd - dd + np.float32(2.0 * MARGIN), 0
    )
    qcnt = int(ok.sum())
    quad = float(np.sum(np.where(ok, ql, 0.0), dtype=np.float64)) / max(qcnt, 1)
    return np.float32(quad if qcnt > 0 else 0.0)


_NC_CACHE = {}


def _get_nc():
    if "nc" not in _NC_CACHE:
        _NC_CACHE["nc"] = _build_nc()
    return _NC_CACHE["nc"]


def _device_consts():
    if "consts" in _NC_CACHE:
        return _NC_CACHE["consts"]
    selg4 = np.zeros((128, 4), np.float32)
    for j in range(4):
        selg4[32 * j : 32 * j + L, j] = 1.0
    egall = np.zeros((128, NG * NG), np.float32)
    for g in range(NG):
        egall[:, g * NG + g] = 1.0
    ones = np.ones((128, 1), np.float32)
    _NC_CACHE["consts"] = (selg4.astype(BF), egall.astype(BF), ones)
    return _NC_CACHE["consts"]


def _build_in_maps(embeddings, classifier_w, classifier_b, labels):
    emb = np.asarray(embeddings, dtype=np.float32).reshape(B * S, H)
    emb_bf = np.ascontiguousarray(emb).astype(BF)
    W = np.asarray(classifier_w, dtype=np.float32)
    b = np.asarray(classifier_b, dtype=np.float32)
    lab_f = np.asarray(labels).reshape(-1).astype(np.int64)

    wt = np.zeros((128, 3 * L), np.float32)
    for c in range(3):
        wt[:, c * L : (c + 1) * L] = W[:, c * 128 : (c + 1) * 128].T
    bcol = np.zeros((128, 1), np.float32)
    for j in range(4):
        bcol[32 * j : 32 * j + L, 0] = b
    selg4, egall, ones = _device_consts()

    in_maps = []
    for cidx in range(NCORES):
        sl = slice(cidx * NTOK, (cidx + 1) * NTOK)
        woh, cews, pairw = _host_tables(lab_f[sl])
        in_maps.append(
            {
                "emb": emb_bf[sl],
                "wt": wt.astype(BF),
                "selg4": selg4,
                "egall": egall,
                "bcol": bcol,
                "woh": woh,
                "cews": cews,
                "pairw": pairw,
                "ones": ones,
            }
        )
    return in_maps, emb, lab_f, b


def kernel(embeddings, classifier_w, classifier_b, labels, attention_mask):
    from concourse.bass_utils import run_bass_kernel_spmd

    in_maps, emb, lab_f, b = _build_in_maps(
        embeddings, classifier_w, classifier_b, labels
    )
    msk_f = np.asarray(attention_mask).reshape(-1).astype(np.int64)
    N = B * S

    nc = _get_nc()
    res = run_bass_kernel_spmd(nc, in_maps, list(range(NCORES)))

    ce_sum = 0.0
    ctx_sum = 0.0
    for cidx in range(NCORES):
        out = res.results[cidx]["outv"]
        ce_sum += float(out[0, 0])
        ctx_sum += float(out[0, 1])

    valid = lab_f != IGNORE
    ce_cnt = int(valid.sum())
    # device sel used logits without bias; correct with sum(cew * b[label])
    lab_safe = np.where(valid, lab_f, 0)
    ce_sum -= float(np.sum(np.where(valid, b[lab_safe], 0.0), dtype=np.float64))
    ce = ce_sum / max(ce_cnt, 1)

    pair_ok = np.zeros(N, dtype=bool)
    kk = np.arange(N - 1)
    in_batch = (kk % S) != (S - 1)
    pair_ok[:-1] = (
        in_batch & (lab_f[:-1] != IGNORE) & (lab_f[:-1] == lab_f[1:]) & (lab_f[:-1] > 0)
    )
    # block-boundary pairs (t % 2048 == 2047) are not covered on device
    t_bound = np.arange(2047, N - 1, 2048)
    t_bound = t_bound[pair_ok[t_bound]]
    if t_bound.size:
        dif = emb[t_bound + 1] - emb[t_bound]
        ctx_sum += float(np.sum(dif * dif, dtype=np.float64))
    pc = int(pair_ok.sum())
    ctx = (ctx_sum / H) / max(pc, 1) if pc > 0 else 0.0

    quad = _quad_host(emb, lab_f, msk_f)

    loss = ce + 0.5 * float(quad) + 0.1 * ctx
    return np.float32(loss)
